# revision 1
# baseline (speedup 1.0000x reference)
"""Trainium2 Bass kernel for nn_MixedLinear (DARTS-style mixed-precision supernet linear).

Reference math (16-term arch-weighted mixture) reduces algebraically to:

  x_mix = C * round(x)                      C = sum(arch_weights)
          [a_scales == 1 and |x| < 7.5, so both activation fake-quant
           branches equal round-half-even(x)]
  w_mix[o,i] = G0(R,Cc)*s0*clip(round(w/s0),-8,7) + G1(R,Cc)*s1*round(w/s1)
          [fake_quant(w * mask) == mask * fake_quant(w); the four (h,it)
           masks collapse into piecewise-constant coefficients over the
           2x2 region grid R = (o >= 3072), Cc = (i >= 768); the 8-bit
           branch's clip never binds for this data]
  out = x_mix @ w_mix^T + beta(R) * bias
      = round(x) @ W_eff^T + b_mix,   W_eff = C * w_mix

Distribution: data-parallel over the 8192 tokens across 8 cores; the
4096x1024 weight is replicated (each core builds the full W_eff on-device).
Each core computes out^T[4096, 1024] with fp32r matmuls (full-rate on the
PE at ~1e-4 relative error; fp32 matmul is 4x slower on trn2), contracting
K=1024 in 8 partition-tiles. Host work is limited to layout (transpose /
shard / concat) and deriving ~12 scalar coefficients from the 16 arch
weights, which are baked into the NEFF as immediates.

Rounding on device uses the magic-number trick: fp32 (v + 1.5*2^23) -
1.5*2^23 == round-half-even(v), matching jnp.round exactly.
"""

import numpy as np

import concourse.mybir as mybir
from concourse import bacc, bass_utils
from concourse.tile import TileContext

N_CORES = 8
B, S, I_DIM, O_DIM = 4, 2048, 1024, 4096
T_TOT = B * S
T_SH = T_TOT // N_CORES  # 1024 tokens per core
NI = I_DIM // 128        # 8 contraction tiles
O_SPAN = 1024            # o-columns per W_eff stage; 3072 boundary aligns
NSP = O_DIM // O_SPAN    # 4 spans (spans 0-2 -> R=0, span 3 -> R=1)
NOT = O_SPAN // 128      # 8 o-tiles per span
TCH = 512                # matmul moving free dim
NTC = T_SH // TCH        # 2 t-chunks
MAGIC = 12582912.0       # 1.5 * 2**23
F32 = mybir.dt.float32
F32R = mybir.dt.float32r
AL = mybir.AluOpType
AF = mybir.ActivationFunctionType

_cache: dict = {}
_last_res = None


def _build(inv_s0, inv_s1, q0, q1, beta0, beta1):
    """Build + compile the per-core kernel. q0/q1 are 2x2 (R, Cc) grids."""
    nc = bacc.Bacc("TRN2", target_bir_lowering=False)
    x_t = nc.dram_tensor("x_t", [I_DIM, T_SH], F32, kind="ExternalInput")
    w_t = nc.dram_tensor("w_t", [I_DIM, O_DIM], F32, kind="ExternalInput")
    b_pt = nc.dram_tensor("b_pt", [128, O_DIM // 128], F32, kind="ExternalInput")
    out_t = nc.dram_tensor("out_t", [O_DIM, T_SH], F32, kind="ExternalOutput")

    with TileContext(nc) as tc:
        with (
            tc.tile_pool(name="px", bufs=1) as px,
            tc.tile_pool(name="pstage", bufs=4) as pstage,
            tc.tile_pool(name="ptmp", bufs=2) as ptmp,
            tc.tile_pool(name="pwe", bufs=2) as pwe,
            tc.tile_pool(name="pout", bufs=12) as pout,
            tc.tile_pool(name="psum", bufs=7, space="PSUM") as psum,
        ):
            # bias columns: b_pt[p, j] = bias[j*128 + p]; j < 24 <=> o < 3072
            bt = pstage.tile([128, O_DIM // 128], F32, tag="bt")
            nc.sync.dma_start(out=bt, in_=b_pt[:, :])
            bs = px.tile([128, O_DIM // 128], F32, tag="bs")
            nc.vector.tensor_scalar(bs[:, 0:24], bt[:, 0:24], float(beta0), None, AL.mult)
            nc.vector.tensor_scalar(bs[:, 24:32], bt[:, 24:32], float(beta1), None, AL.mult)

            # xq[i] = round(x^T tile), exact in fp32r (small integers).
            # Loaded in t-chunk halves: chunk 0 before the first W span so the
            # first matmul chains are not queued behind the full x DMA.
            xq = []
            for i in range(NI):
                q = px.tile([128, T_SH], F32R, tag=f"xq{i}")
                xq.append(q)

            def load_x_chunk(t):
                for i in range(NI):
                    xr = pstage.tile([128, TCH], F32, tag="xr")
                    nc.sync.dma_start(
                        out=xr,
                        in_=x_t[128 * i : 128 * (i + 1), TCH * t : TCH * (t + 1)],
                    )
                    nc.vector.tensor_scalar(
                        xq[i][:, TCH * t : TCH * (t + 1)],
                        xr, MAGIC, MAGIC, AL.add, AL.subtract,
                    )

            load_x_chunk(0)

            for sp in range(NSP):
                R = 1 if sp * O_SPAN >= 3072 else 0
                wes = []
                for i in range(NI):
                    Cc = 1 if i * 128 >= 768 else 0
                    wr = pstage.tile([128, O_SPAN], F32, tag="wr")
                    nc.sync.dma_start(
                        out=wr,
                        in_=w_t[128 * i : 128 * (i + 1), sp * O_SPAN : (sp + 1) * O_SPAN],
                    )
                    # t0 = round(w/s0) + M, t1 = round(w/s1) + M (ACT affine + magic)
                    t0 = ptmp.tile([128, O_SPAN], F32, tag="t0")
                    nc.scalar.activation(t0, wr, AF.Copy, bias=MAGIC, scale=float(inv_s0))
                    t1 = ptmp.tile([128, O_SPAN], F32, tag="t1")
                    nc.scalar.activation(t1, wr, AF.Copy, bias=MAGIC, scale=float(inv_s1))
                    # 4-bit clip in the shifted domain, then exact -M and scale
                    p1 = ptmp.tile([128, O_SPAN], F32, tag="p1")
                    nc.vector.tensor_scalar(p1, t0, MAGIC - 8.0, MAGIC + 7.0, AL.max, AL.min)
                    p2 = ptmp.tile([128, O_SPAN], F32, tag="p2")
                    nc.vector.tensor_scalar(p2, p1, -MAGIC, float(q0[R][Cc]), AL.add, AL.mult)
                    q1t = ptmp.tile([128, O_SPAN], F32, tag="q1")
                    nc.gpsimd.tensor_scalar(q1t, t1, -MAGIC, float(q1[R][Cc]), AL.add, AL.mult)
                    we = pwe.tile([128, O_SPAN], F32R, tag=f"we{i}")
                    nc.vector.tensor_tensor(out=we, in0=p2, in1=q1t, op=AL.add)
                    wes.append(we)

                if sp == 0:
                    for t in range(1, NTC):
                        load_x_chunk(t)

                for t in range(NTC):
                    for ot in range(NOT):
                        og = sp * NOT + ot  # global o-tile index
                        ps = psum.tile([128, TCH], F32, tag="ps")
                        for i in range(NI):
                            nc.tensor.matmul(
                                ps,
                                wes[i][:, 128 * ot : 128 * (ot + 1)],
                                xq[i][:, TCH * t : TCH * (t + 1)],
                                start=(i == 0),
                                stop=(i == NI - 1),
                            )
                        ob = pout.tile([128, TCH], F32, tag="ob")
                        nc.scalar.activation(
                            ob, ps, AF.Identity, bias=bs[:, og : og + 1], scale=1.0
                        )
                        nc.sync.dma_start(
                            out=out_t[og * 128 : (og + 1) * 128, TCH * t : TCH * (t + 1)],
                            in_=ob,
                        )
    nc.compile()
    return nc


def _derive(arch_weights, w_scales):
    aw = np.asarray(arch_weights, dtype=np.float64)
    S4 = aw.reshape(2, 2, 2, 2)  # [h_idx, it_idx, m, n]
    C = float(aw.sum())
    s0 = float(np.asarray(w_scales)[0])  # 4-bit scale
    s1 = float(np.asarray(w_scales)[1])  # 8-bit scale
    Ssum = S4.sum(axis=2)  # [h, it, n]
    G = np.zeros((2, 2, 2))  # [n, R, Cc]
    for n in (0, 1):
        for R in (0, 1):
            its = (0, 1) if R == 0 else (1,)
            for Cc in (0, 1):
                hs = (0, 1) if Cc == 0 else (1,)
                G[n, R, Cc] = sum(Ssum[h, it, n] for it in its for h in hs)
    q0 = (C * G[0] * s0).astype(np.float32)  # [R][Cc]
    q1 = (C * G[1] * s1).astype(np.float32)
    beta0 = np.float32(C)
    beta1 = np.float32(S4[:, 1].sum())
    inv_s0 = np.float32(1.0 / s0)
    inv_s1 = np.float32(1.0 / s1)
    return inv_s0, inv_s1, q0, q1, beta0, beta1, s0, s1


def _fallback(x, arch_weights, weight, bias, a_scales, w_scales):
    """Exact numpy replica of the reference (guard path; not used for the
    shipped input distribution)."""
    aw = np.asarray(arch_weights, np.float32)
    x = np.asarray(x, np.float32)
    w = np.asarray(weight, np.float32)
    b = np.asarray(bias, np.float32)
    a_s = np.asarray(a_scales, np.float32)
    w_s = np.asarray(w_scales, np.float32)
    rows = np.arange(O_DIM)[:, None]
    cols = np.arange(I_DIM)[None, :]

    def fq(v, scale, bit):
        qn, qp = -(2.0 ** (bit - 1)), 2.0 ** (bit - 1) - 1
        return (np.round(np.clip(v / scale, qn, qp)) * scale).astype(np.float32)

    x_mix = np.zeros_like(x)
    w_mix = np.zeros_like(w)
    b_mix = np.zeros_like(b)
    k = 0
    for h in (768, 1024):
        for it in (3072, 4096):
            mask = ((rows < it) & (cols < h)).astype(np.float32)
            w_pad = w * mask
            b_pad = b * (rows[:, 0] < it).astype(np.float32)
            for m, ab in enumerate((4, 8)):
                for n, wb in enumerate((4, 8)):
                    wk = aw[k]
                    x_mix = x_mix + wk * fq(x, a_s[m], ab)
                    w_mix = w_mix + wk * fq(w_pad, w_s[n], wb)
                    b_mix = b_mix + wk * b_pad
                    k += 1
    return (
        np.einsum("bsi,oi->bso", x_mix, w_mix, optimize=True) + b_mix
    ).astype(np.float32)


def _run(inputs, trace=False):
    x = np.ascontiguousarray(np.asarray(inputs["x"], np.float32))
    arch_weights = np.asarray(inputs["arch_weights"], np.float32)
    weight = np.ascontiguousarray(np.asarray(inputs["weight"], np.float32))
    bias = np.ascontiguousarray(np.asarray(inputs["bias"], np.float32))
    a_scales = np.asarray(inputs["a_scales"], np.float32)
    w_scales = np.asarray(inputs["w_scales"], np.float32)

    inv_s0, inv_s1, q0, q1, beta0, beta1, s0, s1 = _derive(arch_weights, w_scales)

    # fast-path validity (always true for the shipped input distribution)
    if not (
        np.all(np.abs(a_scales - 1.0) == 0.0)
        and float(np.abs(x).max()) < 7.49
        and float(np.abs(weight).max()) / s1 < 126.9
    ):
        return _fallback(x, arch_weights, weight, bias, a_scales, w_scales), None

    key = (
        float(inv_s0), float(inv_s1), tuple(q0.ravel().tolist()),
        tuple(q1.ravel().tolist()), float(beta0), float(beta1),
    )
    if key not in _cache:
        _cache.clear()
        _cache[key] = _build(inv_s0, inv_s1, q0, q1, beta0, beta1)
    nc = _cache[key]

    x2 = x.reshape(T_TOT, I_DIM)
    w_tr = np.ascontiguousarray(weight.T)            # [I_DIM, O_DIM]
    b_pt = np.ascontiguousarray(bias.reshape(O_DIM // 128, 128).T)  # [128, 32]
    in_maps = []
    for j in range(N_CORES):
        x_sh = np.ascontiguousarray(x2[j * T_SH : (j + 1) * T_SH].T)  # [I, T_SH]
        in_maps.append({"x_t": x_sh, "w_t": w_tr, "b_pt": b_pt})

    res = bass_utils.run_bass_kernel_spmd(
        nc, in_maps, core_ids=list(range(N_CORES)), trace=trace
    )
    global _last_res
    _last_res = res
    out = np.empty((T_TOT, O_DIM), np.float32)
    for j in range(N_CORES):
        out[j * T_SH : (j + 1) * T_SH] = res.results[j]["out_t"].T
    return out.reshape(B, S, O_DIM), res.exec_time_ns


def kernel(**inputs):
    out, _ = _run(inputs, trace=False)
    return out



# revision 3
# speedup vs baseline: 1.0030x; 1.0030x over previous
"""Trainium2 Bass kernel for nn_MixedLinear (DARTS-style mixed-precision supernet linear).

Reference math (16-term arch-weighted mixture) reduces algebraically to:

  x_mix = C * round(x)                      C = sum(arch_weights)
          [a_scales == 1 and |x| < 7.5, so both activation fake-quant
           branches equal round-half-even(x)]
  w_mix[o,i] = G0(R,Cc)*s0*clip(round(w/s0),-8,7) + G1(R,Cc)*s1*round(w/s1)
          [fake_quant(w * mask) == mask * fake_quant(w); the four (h,it)
           masks collapse into piecewise-constant coefficients over the
           2x2 region grid R = (o >= 3072), Cc = (i >= 768); the 8-bit
           branch's clip never binds for this data]
  out = x_mix @ w_mix^T + beta(R) * bias
      = round(x) @ W_eff^T + b_mix,   W_eff = C * w_mix

Distribution: data-parallel over the 8192 tokens across 8 cores; the
4096x1024 weight is replicated (each core builds the full W_eff on-device).
Each core computes out^T[4096, 1024] with fp32r matmuls (full-rate on the
PE at ~1e-4 relative error; fp32 matmul is 4x slower on trn2), contracting
K=1024 in 8 partition-tiles. Host work is limited to layout (transpose /
shard / concat) and deriving ~12 scalar coefficients from the 16 arch
weights, which are baked into the NEFF as immediates.

Rounding on device uses the magic-number trick: fp32 (v + 1.5*2^23) -
1.5*2^23 == round-half-even(v), matching jnp.round exactly.
"""

import numpy as np

import concourse.mybir as mybir
from concourse import bacc, bass_utils
from concourse.tile import TileContext

N_CORES = 8
B, S, I_DIM, O_DIM = 4, 2048, 1024, 4096
T_TOT = B * S
T_SH = T_TOT // N_CORES  # 1024 tokens per core
NI = I_DIM // 128        # 8 contraction tiles
O_SPAN = 1024            # o-columns per W_eff stage; 3072 boundary aligns
NSP = O_DIM // O_SPAN    # 4 spans (spans 0-2 -> R=0, span 3 -> R=1)
NOT = O_SPAN // 128      # 8 o-tiles per span
TCH = 512                # matmul moving free dim
NTC = T_SH // TCH        # 2 t-chunks
MAGIC = 12582912.0       # 1.5 * 2**23
F32 = mybir.dt.float32
F32R = mybir.dt.float32r
BF16 = mybir.dt.bfloat16
AL = mybir.AluOpType
AF = mybir.ActivationFunctionType

_cache: dict = {}
_last_res = None


def _build(inv_s0, inv_s1, q0, q1, beta0, beta1):
    """Build + compile the per-core kernel. q0/q1 are 2x2 (R, Cc) grids."""
    nc = bacc.Bacc("TRN2", target_bir_lowering=False)
    x_t = nc.dram_tensor("x_t", [I_DIM, T_SH], F32, kind="ExternalInput")
    w_t = nc.dram_tensor("w_t", [I_DIM, O_DIM], F32, kind="ExternalInput")
    b_pt = nc.dram_tensor("b_pt", [128, O_DIM // 128], F32, kind="ExternalInput")
    out_t = nc.dram_tensor("out_t", [O_DIM, T_SH], F32, kind="ExternalOutput")

    with TileContext(nc) as tc:
        with (
            tc.tile_pool(name="px", bufs=1) as px,
            tc.tile_pool(name="pstage", bufs=4) as pstage,
            tc.tile_pool(name="ptmp", bufs=2) as ptmp,
            tc.tile_pool(name="pwe", bufs=2) as pwe,
            tc.tile_pool(name="pout", bufs=12) as pout,
            tc.tile_pool(name="psum", bufs=7, space="PSUM") as psum,
        ):
            # bias columns: b_pt[p, j] = bias[j*128 + p]; j < 24 <=> o < 3072
            bt = pstage.tile([128, O_DIM // 128], F32, tag="bt")
            nc.sync.dma_start(out=bt, in_=b_pt[:, :])
            bs = px.tile([128, O_DIM // 128], F32, tag="bs")
            nc.vector.tensor_scalar(bs[:, 0:24], bt[:, 0:24], float(beta0), None, AL.mult)
            nc.vector.tensor_scalar(bs[:, 24:32], bt[:, 24:32], float(beta1), None, AL.mult)

            # xq[i] = round(x^T tile), exact in fp32r (small integers).
            # Loaded in t-chunk halves: chunk 0 before the first W span so the
            # first matmul chains are not queued behind the full x DMA.
            xq = []
            for i in range(NI):
                q = px.tile([128, T_SH], BF16, tag=f"xq{i}")
                xq.append(q)

            def load_x_chunk(t):
                for i in range(NI):
                    xr = pstage.tile([128, TCH], F32, tag="xr")
                    nc.sync.dma_start(
                        out=xr,
                        in_=x_t[128 * i : 128 * (i + 1), TCH * t : TCH * (t + 1)],
                    )
                    nc.vector.tensor_scalar(
                        xq[i][:, TCH * t : TCH * (t + 1)],
                        xr, MAGIC, MAGIC, AL.add, AL.subtract,
                    )

            load_x_chunk(0)

            for sp in range(NSP):
                R = 1 if sp * O_SPAN >= 3072 else 0
                wes = []
                for i in range(NI):
                    Cc = 1 if i * 128 >= 768 else 0
                    wr = pstage.tile([128, O_SPAN], F32, tag="wr")
                    nc.sync.dma_start(
                        out=wr,
                        in_=w_t[128 * i : 128 * (i + 1), sp * O_SPAN : (sp + 1) * O_SPAN],
                    )
                    # t0 = round(w/s0) + M, t1 = round(w/s1) + M (ACT affine + magic)
                    t0 = ptmp.tile([128, O_SPAN], F32, tag="t0")
                    nc.scalar.activation(t0, wr, AF.Copy, bias=MAGIC, scale=float(inv_s0))
                    t1 = ptmp.tile([128, O_SPAN], F32, tag="t1")
                    nc.scalar.activation(t1, wr, AF.Copy, bias=MAGIC, scale=float(inv_s1))
                    # 4-bit clip in the shifted domain, then exact -M and scale
                    p1 = ptmp.tile([128, O_SPAN], F32, tag="p1")
                    nc.vector.tensor_scalar(p1, t0, MAGIC - 8.0, MAGIC + 7.0, AL.max, AL.min)
                    p2 = ptmp.tile([128, O_SPAN], F32, tag="p2")
                    nc.vector.tensor_scalar(p2, p1, -MAGIC, float(q0[R][Cc]), AL.add, AL.mult)
                    q1t = ptmp.tile([128, O_SPAN], F32, tag="q1")
                    nc.gpsimd.tensor_scalar(q1t, t1, -MAGIC, float(q1[R][Cc]), AL.add, AL.mult)
                    we = pwe.tile([128, O_SPAN], BF16, tag=f"we{i}")
                    nc.vector.tensor_tensor(out=we, in0=p2, in1=q1t, op=AL.add)
                    wes.append(we)

                if sp == 0:
                    for t in range(1, NTC):
                        load_x_chunk(t)

                for t in range(NTC):
                    for ot in range(NOT):
                        og = sp * NOT + ot  # global o-tile index
                        ps = psum.tile([128, TCH], F32, tag="ps")
                        for i in range(NI):
                            nc.tensor.matmul(
                                ps,
                                wes[i][:, 128 * ot : 128 * (ot + 1)],
                                xq[i][:, TCH * t : TCH * (t + 1)],
                                start=(i == 0),
                                stop=(i == NI - 1),
                            )
                        ob = pout.tile([128, TCH], F32, tag="ob")
                        nc.scalar.activation(
                            ob, ps, AF.Identity, bias=bs[:, og : og + 1], scale=1.0
                        )
                        nc.sync.dma_start(
                            out=out_t[og * 128 : (og + 1) * 128, TCH * t : TCH * (t + 1)],
                            in_=ob,
                        )
    nc.compile()
    return nc


def _derive(arch_weights, w_scales):
    aw = np.asarray(arch_weights, dtype=np.float64)
    S4 = aw.reshape(2, 2, 2, 2)  # [h_idx, it_idx, m, n]
    C = float(aw.sum())
    s0 = float(np.asarray(w_scales)[0])  # 4-bit scale
    s1 = float(np.asarray(w_scales)[1])  # 8-bit scale
    Ssum = S4.sum(axis=2)  # [h, it, n]
    G = np.zeros((2, 2, 2))  # [n, R, Cc]
    for n in (0, 1):
        for R in (0, 1):
            its = (0, 1) if R == 0 else (1,)
            for Cc in (0, 1):
                hs = (0, 1) if Cc == 0 else (1,)
                G[n, R, Cc] = sum(Ssum[h, it, n] for it in its for h in hs)
    q0 = (C * G[0] * s0).astype(np.float32)  # [R][Cc]
    q1 = (C * G[1] * s1).astype(np.float32)
    beta0 = np.float32(C)
    beta1 = np.float32(S4[:, 1].sum())
    inv_s0 = np.float32(1.0 / s0)
    inv_s1 = np.float32(1.0 / s1)
    return inv_s0, inv_s1, q0, q1, beta0, beta1, s0, s1


def _fallback(x, arch_weights, weight, bias, a_scales, w_scales):
    """Exact numpy replica of the reference (guard path; not used for the
    shipped input distribution)."""
    aw = np.asarray(arch_weights, np.float32)
    x = np.asarray(x, np.float32)
    w = np.asarray(weight, np.float32)
    b = np.asarray(bias, np.float32)
    a_s = np.asarray(a_scales, np.float32)
    w_s = np.asarray(w_scales, np.float32)
    rows = np.arange(O_DIM)[:, None]
    cols = np.arange(I_DIM)[None, :]

    def fq(v, scale, bit):
        qn, qp = -(2.0 ** (bit - 1)), 2.0 ** (bit - 1) - 1
        return (np.round(np.clip(v / scale, qn, qp)) * scale).astype(np.float32)

    x_mix = np.zeros_like(x)
    w_mix = np.zeros_like(w)
    b_mix = np.zeros_like(b)
    k = 0
    for h in (768, 1024):
        for it in (3072, 4096):
            mask = ((rows < it) & (cols < h)).astype(np.float32)
            w_pad = w * mask
            b_pad = b * (rows[:, 0] < it).astype(np.float32)
            for m, ab in enumerate((4, 8)):
                for n, wb in enumerate((4, 8)):
                    wk = aw[k]
                    x_mix = x_mix + wk * fq(x, a_s[m], ab)
                    w_mix = w_mix + wk * fq(w_pad, w_s[n], wb)
                    b_mix = b_mix + wk * b_pad
                    k += 1
    return (
        np.einsum("bsi,oi->bso", x_mix, w_mix, optimize=True) + b_mix
    ).astype(np.float32)


def _run(inputs, trace=False):
    x = np.ascontiguousarray(np.asarray(inputs["x"], np.float32))
    arch_weights = np.asarray(inputs["arch_weights"], np.float32)
    weight = np.ascontiguousarray(np.asarray(inputs["weight"], np.float32))
    bias = np.ascontiguousarray(np.asarray(inputs["bias"], np.float32))
    a_scales = np.asarray(inputs["a_scales"], np.float32)
    w_scales = np.asarray(inputs["w_scales"], np.float32)

    inv_s0, inv_s1, q0, q1, beta0, beta1, s0, s1 = _derive(arch_weights, w_scales)

    # fast-path validity (always true for the shipped input distribution)
    if not (
        np.all(np.abs(a_scales - 1.0) == 0.0)
        and float(np.abs(x).max()) < 7.49
        and float(np.abs(weight).max()) / s1 < 126.9
    ):
        return _fallback(x, arch_weights, weight, bias, a_scales, w_scales), None

    key = (
        float(inv_s0), float(inv_s1), tuple(q0.ravel().tolist()),
        tuple(q1.ravel().tolist()), float(beta0), float(beta1),
    )
    if key not in _cache:
        _cache.clear()
        _cache[key] = _build(inv_s0, inv_s1, q0, q1, beta0, beta1)
    nc = _cache[key]

    x2 = x.reshape(T_TOT, I_DIM)
    w_tr = np.ascontiguousarray(weight.T)            # [I_DIM, O_DIM]
    b_pt = np.ascontiguousarray(bias.reshape(O_DIM // 128, 128).T)  # [128, 32]
    in_maps = []
    for j in range(N_CORES):
        x_sh = np.ascontiguousarray(x2[j * T_SH : (j + 1) * T_SH].T)  # [I, T_SH]
        in_maps.append({"x_t": x_sh, "w_t": w_tr, "b_pt": b_pt})

    res = bass_utils.run_bass_kernel_spmd(
        nc, in_maps, core_ids=list(range(N_CORES)), trace=trace
    )
    global _last_res
    _last_res = res
    out = np.empty((T_TOT, O_DIM), np.float32)
    for j in range(N_CORES):
        out[j * T_SH : (j + 1) * T_SH] = res.results[j]["out_t"].T
    return out.reshape(B, S, O_DIM), res.exec_time_ns


def kernel(**inputs):
    out, _ = _run(inputs, trace=False)
    return out

